# revision 5
# baseline (speedup 1.0000x reference)
"""Capacity-MoE Trainium2 kernel (8 NeuronCores, expert-parallel).

Contract: kernel(**inputs) takes the FULL inputs of reference.setup_inputs()
and returns the FULL [B, D] float32 output.

Strategy
--------
Host: replicate the reference's capacity-aware routing (a plain cumsum over
the one-hot routes — no feedback loop), build per-expert accepted-token
lists, and shard expert e's tokens (<= capacity) to core e.  Tokens whose
every route overflowed ("dropped") are sharded contiguously across all 8
cores for the fallback MLP.  Device (per core): two dense 2-layer MLP
streams — the core's expert MLP over its [T_pad] gathered tokens and the
fallback MLP over its dropped-token chunk — all activations kept transposed
[D, T] so no on-device transposes are needed.  Host: gather per-assignment
outputs, average by accept count, and patch dropped rows with the fallback.

Matmuls run as float32r (TF32-like reduced-precision fp32, ~1.5e-4 relative
error per matmul, 4x the throughput of true fp32 on the PE array) with fp32
PSUM accumulation.
"""

import os
import sys

for _p in ("/opt/trn_rl_repo",):
    if _p not in sys.path and os.path.isdir(_p):
        sys.path.append(_p)

import numpy as np

import concourse.bass as bass
import concourse.tile as tile
from concourse import mybir
from concourse.bass_utils import run_bass_kernel_spmd

F32 = mybir.dt.float32
DT = mybir.dt.float32r  # matmul operand dtype

D = 1024
NCORES = 8
KCH = 8  # contraction chunks of 128 (D / 128)


# ---------------------------------------------------------------------------
# walrus in this environment rejects instructions with >1 sync wait; split
# extra waits onto same-engine NoOps inserted directly before the offender.
def _split_multi_waits(nc):
    ctr = 0
    for f in nc.m.functions:
        for bb in f.blocks:
            il = bb.instructions
            i = 0
            while i < len(il):
                inst = il[i]
                si = inst.sync_info
                if si is None or si.on_wait is None or len(si.on_wait) <= 1:
                    i += 1
                    continue
                waits = list(si.on_wait)
                for w in waits[:-1]:
                    ctr += 1
                    nop = mybir.InstNoOp(name=f"waitsplit-{ctr}")
                    nop.engine = inst.engine
                    nop.sync_info = mybir.SyncInfo(on_wait=[w], on_update=[])
                    il.insert(i, nop)
                    i += 1
                inst.sync_info = mybir.SyncInfo(
                    on_wait=[waits[-1]], on_update=list(si.on_update or [])
                )
                i += 1
    return nc


def _ntiles(T):
    out, off = [], 0
    while off < T:
        n = min(512, T - off)
        out.append((off, n))
        off += n
    return out


def _build(T_pad, F_pad):
    nc = bass.Bass()

    xT = nc.dram_tensor("xT", [D, T_pad], DT, kind="ExternalInput")
    w1T = nc.dram_tensor("w1T", [D, D], DT, kind="ExternalInput")
    b1 = nc.dram_tensor("b1", [128, KCH], F32, kind="ExternalInput")
    w2T = nc.dram_tensor("w2T", [D, D], DT, kind="ExternalInput")
    b2 = nc.dram_tensor("b2", [128, KCH], F32, kind="ExternalInput")
    xfT = nc.dram_tensor("xfT", [D, F_pad], DT, kind="ExternalInput")
    wf1T = nc.dram_tensor("wf1T", [D, D], DT, kind="ExternalInput")
    bf1 = nc.dram_tensor("bf1", [128, KCH], F32, kind="ExternalInput")
    wf2T = nc.dram_tensor("wf2T", [D, D], DT, kind="ExternalInput")
    bf2 = nc.dram_tensor("bf2", [128, KCH], F32, kind="ExternalInput")
    yT = nc.dram_tensor("yT", [D, T_pad], F32, kind="ExternalOutput")
    yfT = nc.dram_tensor("yfT", [D, F_pad], F32, kind="ExternalOutput")

    Relu = mybir.ActivationFunctionType.Relu
    Ident = mybir.ActivationFunctionType.Identity

    with tile.TileContext(nc) as tc:
        with tc.tile_pool(name="cp", bufs=1) as cp, \
             tc.tile_pool(name="xp", bufs=1) as xp, \
             tc.tile_pool(name="hp", bufs=1) as hp, \
             tc.tile_pool(name="wp", bufs=2) as wp, \
             tc.tile_pool(name="yp", bufs=3) as yp, \
             tc.tile_pool(name="pp", bufs=8, space="PSUM") as pp:

            biases = {}
            for name, dram in (("b1", b1), ("b2", b2),
                               ("bf1", bf1), ("bf2", bf2)):
                t = cp.tile([128, KCH], F32, tag=name, name=name)
                nc.sync.dma_start(out=t, in_=dram[:, :])
                biases[name] = t

            def alloc_kchunks(T, tagp, pool, dt=DT):
                return [pool.tile([128, T], dt, tag=f"{tagp}{k}",
                                  name=f"{tagp}{k}")
                        for k in range(KCH)]

            def load_interleaved(pairs):
                # (tile, dram) pairs in consumption order
                for t, dram, k in pairs:
                    nc.sync.dma_start(out=t, in_=dram[k * 128:(k + 1) * 128, :])

            # SBUF tiles.  Weight tags w0..w7 rotate through 2 slots each:
            # wf1 -> w1 -> w2 -> wf2 in allocation order, so each layer's
            # weight DMA can prefetch while the previous layer computes.
            xfs = alloc_kchunks(F_pad, "xf", xp)
            wsf1 = alloc_kchunks(D, "w", wp)
            xs = alloc_kchunks(T_pad, "x", xp)
            ws1 = alloc_kchunks(D, "w", wp)

            # fallback inputs land first (small), expert stream right behind;
            # interleave (weight-chunk, x-chunk) pairs in consumption order so
            # the PE's k-loop can start as soon as the first pair arrives
            pairs = []
            for k in range(KCH):
                pairs += [(wsf1[k], wf1T, k), (xfs[k], xfT, k)]
            for k in range(KCH):
                pairs += [(ws1[k], w1T, k), (xs[k], xT, k)]
            load_interleaved(pairs)

            def layer_kouter(src, ws, T, mgrp, out_cb):
                """k-outer accumulation: psum group = mgrp m-chunks x all
                n-tiles; weights each loaded once per (k, m); PE can start
                as soon as the first (w_k, x_k) chunk pair lands."""
                nt = _ntiles(T)
                for mg in range(0, KCH, mgrp):
                    pss = {(m, off): pp.tile([128, n], F32, tag="ps", name="ps")
                           for m in range(mg, mg + mgrp) for off, n in nt}
                    for k in range(KCH):
                        for m in range(mg, mg + mgrp):
                            for off, n in nt:
                                nc.tensor.matmul(
                                    pss[(m, off)],
                                    ws[k][:, m * 128:(m + 1) * 128],
                                    src[k][:, off:off + n],
                                    start=(k == 0),
                                    stop=(k == KCH - 1),
                                )
                    for m in range(mg, mg + mgrp):
                        for off, n in nt:
                            out_cb(m, off, n, pss[(m, off)])

            def to_h(hs, b1t):
                def cb(m, off, n, ps):
                    nc.scalar.activation(hs[m][:, off:off + n], ps, Relu,
                                         bias=b1t[:, m:m + 1])
                return cb

            def to_y(ydram, b2t, ytag):
                def cb(m, off, n, ps):
                    yt = yp.tile([128, n], F32, tag=ytag, name=ytag)
                    nc.scalar.activation(yt, ps, Ident, bias=b2t[:, m:m + 1])
                    nc.sync.dma_start(
                        out=ydram[m * 128:(m + 1) * 128, off:off + n], in_=yt)
                return cb

            # fallback L1 (small, data arrives first -> fills the DMA ramp)
            hfs = alloc_kchunks(F_pad, "hf", hp)
            layer_kouter(xfs, wsf1, F_pad, 4, to_h(hfs, biases["bf1"]))

            # expert L1, then L2
            hs = alloc_kchunks(T_pad, "h", hp)
            layer_kouter(xs, ws1, T_pad, 2, to_h(hs, biases["b1"]))

            ws2 = alloc_kchunks(D, "w", wp)
            load_interleaved([(ws2[k], w2T, k) for k in range(KCH)])
            layer_kouter(hs, ws2, T_pad, 2, to_y(yT, biases["b2"], "y"))

            # fallback L2
            wsf2 = alloc_kchunks(D, "w", wp)
            load_interleaved([(wsf2[k], wf2T, k) for k in range(KCH)])
            layer_kouter(hfs, wsf2, F_pad, 4, to_y(yfT, biases["bf2"], "yf"))

    _split_multi_waits(nc)
    return nc


_NC_CACHE = {}


def _get_nc(T_pad, F_pad):
    key = (T_pad, F_pad)
    if key not in _NC_CACHE:
        _NC_CACHE[key] = _build(T_pad, F_pad)
    return _NC_CACHE[key]


def _round_up(v, m):
    return ((v + m - 1) // m) * m


def kernel(x, W1, b1, W2, b2, Wf1, bf1, Wf2, bf2, routes, capacity,
           _trace=False):
    x = np.ascontiguousarray(np.asarray(x, dtype=np.float32))
    W1 = np.asarray(W1, dtype=np.float32)
    b1 = np.asarray(b1, dtype=np.float32)
    W2 = np.asarray(W2, dtype=np.float32)
    b2 = np.asarray(b2, dtype=np.float32)
    Wf1 = np.asarray(Wf1, dtype=np.float32)
    bf1 = np.asarray(bf1, dtype=np.float32)
    Wf2 = np.asarray(Wf2, dtype=np.float32)
    bf2 = np.asarray(bf2, dtype=np.float32)
    routes = np.asarray(routes)
    capacity = int(np.asarray(capacity))

    B, Dm = x.shape
    E = W1.shape[0]
    Kk = routes.shape[1]
    assert Dm == D and E == NCORES

    # --- routing: exact reference semantics (vectorized cumsum) ---
    e = routes.reshape(-1).astype(np.int64)
    valid = (e >= 0) & (e < E)
    e_safe = np.where(valid, e, 0)
    idx = np.arange(B * Kk)
    oh = np.zeros((B * Kk, E), dtype=np.int32)
    oh[idx[valid], e[valid]] = 1
    rank = np.cumsum(oh, axis=0) - oh
    rank_at = rank[idx, e_safe]
    accept_flat = valid & (rank_at < capacity)
    used = accept_flat.reshape(B, Kk).sum(1)

    # per-expert accepted assignment lists (flat order == reference order)
    tok_lists, fidx_lists, counts = [], [], []
    for el in range(E):
        fidx = np.nonzero(accept_flat & (e_safe == el))[0]
        fidx_lists.append(fidx)
        tok_lists.append(fidx // Kk)
        counts.append(len(fidx))
    T_pad = max(256, _round_up(max(counts), 256))
    src_flat = np.full(B * Kk, -1, dtype=np.int64)
    for el in range(E):
        src_flat[fidx_lists[el]] = el * T_pad + np.arange(counts[el])

    dropped = np.nonzero(used == 0)[0]
    F = len(dropped)
    Fc = max(1, -(-F // NCORES))
    F_pad = max(128, _round_up(Fc, 128))

    nc = _get_nc(T_pad, F_pad)

    # --- shard inputs ---
    def btile(v):
        return np.ascontiguousarray(v.reshape(KCH, 128).T)

    in_maps = []
    shared = {}
    for el in range(E):
        shared_el = {
            "w1T": np.ascontiguousarray(W1[el].T),
            "b1": btile(b1[el]),
            "w2T": np.ascontiguousarray(W2[el].T),
            "b2": btile(b2[el]),
        }
        if "wf1T" not in shared:
            shared["wf1T"] = np.ascontiguousarray(Wf1.T)
            shared["bf1"] = btile(bf1)
            shared["wf2T"] = np.ascontiguousarray(Wf2.T)
            shared["bf2"] = btile(bf2)

        toks = tok_lists[el]
        tpad = np.zeros(T_pad, dtype=np.int64)
        tpad[:len(toks)] = toks
        xTe = np.ascontiguousarray(x[tpad].T)

        lo, hi = el * Fc, min((el + 1) * Fc, F)
        fpad = np.zeros(F_pad, dtype=np.int64)
        if hi > lo:
            fpad[:hi - lo] = dropped[lo:hi]
        xfTe = np.ascontiguousarray(x[fpad].T)

        in_maps.append({"xT": xTe, "xfT": xfTe, **shared_el, **shared})

    res = run_bass_kernel_spmd(nc, in_maps, core_ids=list(range(NCORES)),
                               trace=_trace)

    # --- combine ---
    G = np.zeros((E * T_pad + 1, D), dtype=np.float32)
    for el in range(E):
        G[el * T_pad:(el + 1) * T_pad] = res.results[el]["yT"].T
    src = np.where(src_flat >= 0, src_flat, E * T_pad).reshape(B, Kk)
    summed = G[src].sum(axis=1)
    out = summed / np.maximum(used, 1.0).astype(np.float32)[:, None]

    if F > 0:
        fb_rows = np.empty((F, D), dtype=np.float32)
        for el in range(E):
            lo, hi = el * Fc, min((el + 1) * Fc, F)
            if hi > lo:
                fb_rows[lo:hi] = res.results[el]["yfT"].T[:hi - lo]
        out[dropped] = fb_rows

    if _trace:
        return out, res
    return out


# revision 8
# speedup vs baseline: 1.0334x; 1.0334x over previous
"""Capacity-MoE Trainium2 kernel (8 NeuronCores, expert-parallel).

Contract: kernel(**inputs) takes the FULL inputs of reference.setup_inputs()
and returns the FULL [B, D] float32 output.

Strategy
--------
Host: replicate the reference's capacity-aware routing (a plain cumsum over
the one-hot routes — no feedback loop), build per-expert accepted-token
lists, and shard expert e's tokens (<= capacity) to core e.  Tokens whose
every route overflowed ("dropped") are sharded contiguously across all 8
cores for the fallback MLP.  Device (per core): two dense 2-layer MLP
streams — the core's expert MLP over its [T_pad] gathered tokens and the
fallback MLP over its dropped-token chunk — all activations kept transposed
[D, T] so no on-device transposes are needed.  Host: gather per-assignment
outputs, average by accept count, and patch dropped rows with the fallback.

Matmuls run as float32r (TF32-like reduced-precision fp32, ~1.5e-4 relative
error per matmul, 4x the throughput of true fp32 on the PE array) with fp32
PSUM accumulation.
"""

import os
import sys

for _p in ("/opt/trn_rl_repo",):
    if _p not in sys.path and os.path.isdir(_p):
        sys.path.append(_p)

import numpy as np

import concourse.bass as bass
import concourse.tile as tile
from concourse import mybir
from concourse.bass_utils import run_bass_kernel_spmd

F32 = mybir.dt.float32
DT = mybir.dt.float32r  # matmul operand dtype

D = 1024
NCORES = 8
KCH = 8  # contraction chunks of 128 (D / 128)


# ---------------------------------------------------------------------------
# walrus in this environment rejects instructions with >1 sync wait; split
# extra waits onto same-engine NoOps inserted directly before the offender.
def _split_multi_waits(nc):
    ctr = 0
    for f in nc.m.functions:
        for bb in f.blocks:
            il = bb.instructions
            i = 0
            while i < len(il):
                inst = il[i]
                si = inst.sync_info
                if si is None or si.on_wait is None or len(si.on_wait) <= 1:
                    i += 1
                    continue
                waits = list(si.on_wait)
                for w in waits[:-1]:
                    ctr += 1
                    nop = mybir.InstNoOp(name=f"waitsplit-{ctr}")
                    nop.engine = inst.engine
                    nop.sync_info = mybir.SyncInfo(on_wait=[w], on_update=[])
                    il.insert(i, nop)
                    i += 1
                inst.sync_info = mybir.SyncInfo(
                    on_wait=[waits[-1]], on_update=list(si.on_update or [])
                )
                i += 1
    return nc


def _ntiles(T):
    out, off = [], 0
    while off < T:
        n = min(512, T - off)
        out.append((off, n))
        off += n
    return out


def _build(T_pad, F_pad):
    nc = bass.Bass()

    xT = nc.dram_tensor("xT", [D, T_pad], DT, kind="ExternalInput")
    w1T = nc.dram_tensor("w1T", [D, D], DT, kind="ExternalInput")
    b1 = nc.dram_tensor("b1", [128, KCH], F32, kind="ExternalInput")
    w2T = nc.dram_tensor("w2T", [D, D], DT, kind="ExternalInput")
    b2 = nc.dram_tensor("b2", [128, KCH], F32, kind="ExternalInput")
    xfT = nc.dram_tensor("xfT", [D, F_pad], DT, kind="ExternalInput")
    wf1T = nc.dram_tensor("wf1T", [D, D], DT, kind="ExternalInput")
    bf1 = nc.dram_tensor("bf1", [128, KCH], F32, kind="ExternalInput")
    wf2T = nc.dram_tensor("wf2T", [D, D], DT, kind="ExternalInput")
    bf2 = nc.dram_tensor("bf2", [128, KCH], F32, kind="ExternalInput")
    yT = nc.dram_tensor("yT", [D, T_pad], F32, kind="ExternalOutput")
    yfT = nc.dram_tensor("yfT", [D, F_pad], F32, kind="ExternalOutput")

    Relu = mybir.ActivationFunctionType.Relu
    Ident = mybir.ActivationFunctionType.Identity

    with tile.TileContext(nc) as tc:
        with tc.tile_pool(name="cp", bufs=1) as cp, \
             tc.tile_pool(name="xp", bufs=1) as xp, \
             tc.tile_pool(name="hp", bufs=1) as hp, \
             tc.tile_pool(name="wp", bufs=2) as wp, \
             tc.tile_pool(name="yp", bufs=3) as yp, \
             tc.tile_pool(name="pp", bufs=8, space="PSUM") as pp:

            biases = {}
            for name, dram in (("b1", b1), ("b2", b2),
                               ("bf1", bf1), ("bf2", bf2)):
                t = cp.tile([128, KCH], F32, tag=name, name=name)
                # gpsimd queue: keep the tiny bias loads off the Sync
                # descriptor stream that feeds the PE-critical pair loads
                nc.gpsimd.dma_start(out=t, in_=dram[:, :])
                biases[name] = t

            def alloc_kchunks(T, tagp, pool, dt=DT):
                return [pool.tile([128, T], dt, tag=f"{tagp}{k}",
                                  name=f"{tagp}{k}")
                        for k in range(KCH)]

            def load_interleaved(pairs):
                # (tile, dram) pairs in consumption order
                for t, dram, k in pairs:
                    nc.sync.dma_start(out=t, in_=dram[k * 128:(k + 1) * 128, :])

            # SBUF tiles.  Weight tags w0..w7 rotate through 2 slots each:
            # wf1 -> w1 -> w2 -> wf2 in allocation order, so each layer's
            # weight DMA can prefetch while the previous layer computes.
            xfs = alloc_kchunks(F_pad, "xf", xp)
            wsf1 = alloc_kchunks(D, "w", wp)
            xs = alloc_kchunks(T_pad, "x", xp)
            ws1 = alloc_kchunks(D, "w", wp)

            # fallback inputs land first (small), expert stream right behind;
            # interleave (weight-chunk, x-chunk) pairs in consumption order so
            # the PE's k-loop can start as soon as the first pair arrives
            pairs = []
            for k in range(KCH):
                pairs += [(wsf1[k], wf1T, k), (xfs[k], xfT, k)]
            for k in range(KCH):
                pairs += [(ws1[k], w1T, k), (xs[k], xT, k)]
            load_interleaved(pairs)

            def layer_kouter(src, ws, T, out_cb):
                """DMA-paced phase: one n-tile x all 8 m-chunks per PSUM
                group (8 banks), k outer — maximizes PE work per arriving
                (w_k, x_k) chunk pair."""
                for off, n in _ntiles(T):
                    pss = [pp.tile([128, n], F32, tag="ps", name="ps")
                           for _ in range(KCH)]
                    for k in range(KCH):
                        for m in range(KCH):
                            nc.tensor.matmul(
                                pss[m],
                                ws[k][:, m * 128:(m + 1) * 128],
                                src[k][:, off:off + n],
                                start=(k == 0),
                                stop=(k == KCH - 1),
                            )
                    for m in range(KCH):
                        out_cb(m, off, n, pss[m])

            def layer_wreuse(src, ws, T, out_cb):
                """Compute-bound phase: m outer, k mid, n inner — each
                stationary weight tile loaded once per (m, k) and reused
                across all n-tiles (3 PSUM banks live)."""
                nt = _ntiles(T)
                for m in range(KCH):
                    pss = {off: pp.tile([128, n], F32, tag="ps", name="ps")
                           for off, n in nt}
                    for k in range(KCH):
                        for off, n in nt:
                            nc.tensor.matmul(
                                pss[off],
                                ws[k][:, m * 128:(m + 1) * 128],
                                src[k][:, off:off + n],
                                start=(k == 0),
                                stop=(k == KCH - 1),
                            )
                    for off, n in nt:
                        out_cb(m, off, n, pss[off])

            def to_h(hs, b1t):
                def cb(m, off, n, ps):
                    nc.scalar.activation(hs[m][:, off:off + n], ps, Relu,
                                         bias=b1t[:, m:m + 1])
                return cb

            def to_y(ydram, b2t, ytag):
                def cb(m, off, n, ps):
                    yt = yp.tile([128, n], F32, tag=ytag, name=ytag)
                    nc.scalar.activation(yt, ps, Ident, bias=b2t[:, m:m + 1])
                    nc.sync.dma_start(
                        out=ydram[m * 128:(m + 1) * 128, off:off + n], in_=yt)
                return cb

            # fallback L1 (small, data arrives first -> fills the DMA ramp)
            hfs = alloc_kchunks(F_pad, "hf", hp)
            layer_kouter(xfs, wsf1, F_pad, to_h(hfs, biases["bf1"]))

            # expert L1 (still DMA-paced)
            hs = alloc_kchunks(T_pad, "h", hp)
            layer_kouter(xs, ws1, T_pad, to_h(hs, biases["b1"]))

            ws2 = alloc_kchunks(D, "w", wp)
            load_interleaved([(ws2[k], w2T, k) for k in range(KCH)])
            wsf2 = alloc_kchunks(D, "w", wp)
            load_interleaved([(wsf2[k], wf2T, k) for k in range(KCH)])

            # fallback L2 before expert L2: the kernel tail is then the
            # (small) last expert chunk's ACT + out-DMA, not a whole
            # fallback stream
            layer_wreuse(hfs, wsf2, F_pad, to_y(yfT, biases["bf2"], "yf"))
            layer_wreuse(hs, ws2, T_pad, to_y(yT, biases["b2"], "y"))

    _split_multi_waits(nc)
    return nc


_NC_CACHE = {}


def _get_nc(T_pad, F_pad):
    key = (T_pad, F_pad)
    if key not in _NC_CACHE:
        _NC_CACHE[key] = _build(T_pad, F_pad)
    return _NC_CACHE[key]


def _round_up(v, m):
    return ((v + m - 1) // m) * m


def kernel(x, W1, b1, W2, b2, Wf1, bf1, Wf2, bf2, routes, capacity,
           _trace=False):
    x = np.ascontiguousarray(np.asarray(x, dtype=np.float32))
    W1 = np.asarray(W1, dtype=np.float32)
    b1 = np.asarray(b1, dtype=np.float32)
    W2 = np.asarray(W2, dtype=np.float32)
    b2 = np.asarray(b2, dtype=np.float32)
    Wf1 = np.asarray(Wf1, dtype=np.float32)
    bf1 = np.asarray(bf1, dtype=np.float32)
    Wf2 = np.asarray(Wf2, dtype=np.float32)
    bf2 = np.asarray(bf2, dtype=np.float32)
    routes = np.asarray(routes)
    capacity = int(np.asarray(capacity))

    B, Dm = x.shape
    E = W1.shape[0]
    Kk = routes.shape[1]
    assert Dm == D and E == NCORES

    # --- routing: exact reference semantics (vectorized cumsum) ---
    e = routes.reshape(-1).astype(np.int64)
    valid = (e >= 0) & (e < E)
    e_safe = np.where(valid, e, 0)
    idx = np.arange(B * Kk)
    oh = np.zeros((B * Kk, E), dtype=np.int32)
    oh[idx[valid], e[valid]] = 1
    rank = np.cumsum(oh, axis=0) - oh
    rank_at = rank[idx, e_safe]
    accept_flat = valid & (rank_at < capacity)
    used = accept_flat.reshape(B, Kk).sum(1)

    # per-expert accepted assignment lists (flat order == reference order)
    tok_lists, fidx_lists, counts = [], [], []
    for el in range(E):
        fidx = np.nonzero(accept_flat & (e_safe == el))[0]
        fidx_lists.append(fidx)
        tok_lists.append(fidx // Kk)
        counts.append(len(fidx))
    T_pad = max(256, _round_up(max(counts), 256))
    src_flat = np.full(B * Kk, -1, dtype=np.int64)
    for el in range(E):
        src_flat[fidx_lists[el]] = el * T_pad + np.arange(counts[el])

    dropped = np.nonzero(used == 0)[0]
    F = len(dropped)
    Fc = max(1, -(-F // NCORES))
    F_pad = max(128, _round_up(Fc, 128))

    nc = _get_nc(T_pad, F_pad)

    # --- shard inputs ---
    def btile(v):
        return np.ascontiguousarray(v.reshape(KCH, 128).T)

    in_maps = []
    shared = {}
    for el in range(E):
        shared_el = {
            "w1T": np.ascontiguousarray(W1[el].T),
            "b1": btile(b1[el]),
            "w2T": np.ascontiguousarray(W2[el].T),
            "b2": btile(b2[el]),
        }
        if "wf1T" not in shared:
            shared["wf1T"] = np.ascontiguousarray(Wf1.T)
            shared["bf1"] = btile(bf1)
            shared["wf2T"] = np.ascontiguousarray(Wf2.T)
            shared["bf2"] = btile(bf2)

        toks = tok_lists[el]
        tpad = np.zeros(T_pad, dtype=np.int64)
        tpad[:len(toks)] = toks
        xTe = np.ascontiguousarray(x[tpad].T)

        lo, hi = el * Fc, min((el + 1) * Fc, F)
        fpad = np.zeros(F_pad, dtype=np.int64)
        if hi > lo:
            fpad[:hi - lo] = dropped[lo:hi]
        xfTe = np.ascontiguousarray(x[fpad].T)

        in_maps.append({"xT": xTe, "xfT": xfTe, **shared_el, **shared})

    res = run_bass_kernel_spmd(nc, in_maps, core_ids=list(range(NCORES)),
                               trace=_trace)

    # --- combine ---
    G = np.zeros((E * T_pad + 1, D), dtype=np.float32)
    for el in range(E):
        G[el * T_pad:(el + 1) * T_pad] = res.results[el]["yT"].T
    src = np.where(src_flat >= 0, src_flat, E * T_pad).reshape(B, Kk)
    summed = G[src].sum(axis=1)
    out = summed / np.maximum(used, 1.0).astype(np.float32)[:, None]

    if F > 0:
        fb_rows = np.empty((F, D), dtype=np.float32)
        for el in range(E):
            lo, hi = el * Fc, min((el + 1) * Fc, F)
            if hi > lo:
                fb_rows[lo:hi] = res.results[el]["yfT"].T[:hi - lo]
        out[dropped] = fb_rows

    if _trace:
        return out, res
    return out


# revision 10
# speedup vs baseline: 1.1792x; 1.1411x over previous
"""Capacity-MoE Trainium2 kernel (8 NeuronCores, expert-parallel).

Contract: kernel(**inputs) takes the FULL inputs of reference.setup_inputs()
and returns the FULL [B, D] float32 output.

Strategy
--------
Host: replicate the reference's capacity-aware routing (a plain cumsum over
the one-hot routes — no feedback loop), build per-expert accepted-token
lists, and shard expert e's tokens (<= capacity) to core e.  Tokens whose
every route overflowed ("dropped") are sharded contiguously across all 8
cores for the fallback MLP.  Device (per core): two dense 2-layer MLP
streams — the core's expert MLP over its [T_pad] gathered tokens and the
fallback MLP over its dropped-token chunk — all activations kept transposed
[D, T] so no on-device transposes are needed.  Host: gather per-assignment
outputs, average by accept count, and patch dropped rows with the fallback.

Matmuls run as float32r (TF32-like reduced-precision fp32, ~1.5e-4 relative
error per matmul, 4x the throughput of true fp32 on the PE array) with fp32
PSUM accumulation.
"""

import os
import sys

for _p in ("/opt/trn_rl_repo",):
    if _p not in sys.path and os.path.isdir(_p):
        sys.path.append(_p)

import numpy as np

import concourse.bass as bass
import concourse.tile as tile
from concourse import mybir
from concourse.bass_utils import run_bass_kernel_spmd

F32 = mybir.dt.float32
DT = mybir.dt.float32r  # matmul operand dtype

D = 1024
NCORES = 8
KCH = 8  # contraction chunks of 128 (D / 128)


# ---------------------------------------------------------------------------
# walrus in this environment rejects instructions with >1 sync wait; split
# extra waits onto same-engine NoOps inserted directly before the offender.
def _split_multi_waits(nc):
    ctr = 0
    for f in nc.m.functions:
        for bb in f.blocks:
            il = bb.instructions
            i = 0
            while i < len(il):
                inst = il[i]
                si = inst.sync_info
                if si is None or si.on_wait is None or len(si.on_wait) <= 1:
                    i += 1
                    continue
                waits = list(si.on_wait)
                for w in waits[:-1]:
                    ctr += 1
                    nop = mybir.InstNoOp(name=f"waitsplit-{ctr}")
                    nop.engine = inst.engine
                    nop.sync_info = mybir.SyncInfo(on_wait=[w], on_update=[])
                    il.insert(i, nop)
                    i += 1
                inst.sync_info = mybir.SyncInfo(
                    on_wait=[waits[-1]], on_update=list(si.on_update or [])
                )
                i += 1
    return nc


def _ntiles(T):
    out, off = [], 0
    while off < T:
        n = min(512, T - off)
        out.append((off, n))
        off += n
    return out


def _build(T_pad, F_pad):
    nc = bass.Bass()

    xT = nc.dram_tensor("xT", [D, T_pad], DT, kind="ExternalInput")
    w1T = nc.dram_tensor("w1T", [D, D], DT, kind="ExternalInput")
    b1 = nc.dram_tensor("b1", [128, KCH], F32, kind="ExternalInput")
    w2T = nc.dram_tensor("w2T", [D, D], DT, kind="ExternalInput")
    b2 = nc.dram_tensor("b2", [128, KCH], F32, kind="ExternalInput")
    xfT = nc.dram_tensor("xfT", [D, F_pad], DT, kind="ExternalInput")
    wf1T = nc.dram_tensor("wf1T", [D, D], DT, kind="ExternalInput")
    bf1 = nc.dram_tensor("bf1", [128, KCH], F32, kind="ExternalInput")
    wf2T = nc.dram_tensor("wf2T", [D, D], DT, kind="ExternalInput")
    bf2 = nc.dram_tensor("bf2", [128, KCH], F32, kind="ExternalInput")
    yT = nc.dram_tensor("yT", [D, T_pad], F32, kind="ExternalOutput")
    yfT = nc.dram_tensor("yfT", [D, F_pad], F32, kind="ExternalOutput")

    Relu = mybir.ActivationFunctionType.Relu
    Ident = mybir.ActivationFunctionType.Identity

    with tile.TileContext(nc) as tc:
        with tc.tile_pool(name="cp", bufs=1) as cp, \
             tc.tile_pool(name="xp", bufs=1) as xp, \
             tc.tile_pool(name="hp", bufs=1) as hp, \
             tc.tile_pool(name="wp", bufs=1) as wp, \
             tc.tile_pool(name="yp", bufs=3) as yp, \
             tc.tile_pool(name="pp", bufs=8, space="PSUM") as pp:

            biases = {}
            for name, dram in (("b1", b1), ("b2", b2),
                               ("bf1", bf1), ("bf2", bf2)):
                t = cp.tile([128, KCH], F32, tag=name, name=name)
                # gpsimd queue: keep the tiny bias loads off the Sync
                # descriptor stream that feeds the PE-critical pair loads
                nc.gpsimd.dma_start(out=t, in_=dram[:, :])
                biases[name] = t

            def alloc_kchunks(T, tagp, pool, dt=DT):
                return [pool.tile([128, T], dt, tag=f"{tagp}{k}",
                                  name=f"{tagp}{k}")
                        for k in range(KCH)]

            def load_kchunks(ts, dram):
                for k in range(KCH):
                    nc.sync.dma_start(out=ts[k],
                                      in_=dram[k * 128:(k + 1) * 128, :])

            nt = _ntiles(T_pad)
            ntf = _ntiles(F_pad)

            # x token-blocks: separate tiles per (k, block) so each block's
            # matmuls depend only on that block's DMA
            xs = [[xp.tile([128, n], DT, tag=f"x{k}b{bi}", name=f"x{k}b{bi}")
                   for bi, (off, n) in enumerate(nt)] for k in range(KCH)]
            xfs = [[xp.tile([128, n], DT, tag=f"x{k}b0", name=f"xf{k}")
                    for (off, n) in ntf[:1]] for k in range(KCH)]
            ws1 = alloc_kchunks(D, "w1", wp)
            ws2 = alloc_kchunks(D, "w2", wp)
            wsf1 = alloc_kchunks(D, "w1", wp)  # reuses w1 slots after L1
            wsf2 = alloc_kchunks(D, "w2", wp)  # reuses w2 slots after L2

            # DMA order = consumption order:
            # (w1_k, x_k[block0]) pairs -> PE starts after the first pair;
            # then remaining token blocks (each enables a full 8-bank sweep);
            # then w2 (for L2), fallback inputs, fallback L2 weights.
            for k in range(KCH):
                nc.sync.dma_start(out=ws1[k],
                                  in_=w1T[k * 128:(k + 1) * 128, :])
                off, n = nt[0]
                nc.sync.dma_start(out=xs[k][0],
                                  in_=xT[k * 128:(k + 1) * 128, off:off + n])
            for bi, (off, n) in enumerate(nt[1:], start=1):
                for k in range(KCH):
                    nc.sync.dma_start(
                        out=xs[k][bi],
                        in_=xT[k * 128:(k + 1) * 128, off:off + n])
            load_kchunks(ws2, w2T)
            for k in range(KCH):
                nc.sync.dma_start(out=wsf1[k],
                                  in_=wf1T[k * 128:(k + 1) * 128, :])
                off, n = ntf[0]
                nc.sync.dma_start(out=xfs[k][0],
                                  in_=xfT[k * 128:(k + 1) * 128, off:off + n])
            load_kchunks(wsf2, wf2T)

            def layer_blocked(src, ws, ntl, out_cb):
                """L1 form: per token-block, full 8-bank PSUM sweep
                (8 m-chunks), k accumulation inner."""
                for bi, (off, n) in enumerate(ntl):
                    pss = [pp.tile([128, n], F32, tag="ps", name="ps")
                           for _ in range(KCH)]
                    for k in range(KCH):
                        for m in range(KCH):
                            nc.tensor.matmul(
                                pss[m],
                                ws[k][:, m * 128:(m + 1) * 128],
                                src[k][bi],
                                start=(k == 0),
                                stop=(k == KCH - 1),
                            )
                    for m in range(KCH):
                        out_cb(m, off, n, pss[m])

            def layer_wreuse(src, ws, ntl, out_cb):
                """L2 form: m outer, k mid, block inner — each stationary
                weight tile loaded once per (m, k), reused across blocks."""
                for m in range(KCH):
                    pss = {off: pp.tile([128, n], F32, tag="ps", name="ps")
                           for off, n in ntl}
                    for k in range(KCH):
                        for off, n in ntl:
                            nc.tensor.matmul(
                                pss[off],
                                ws[k][:, m * 128:(m + 1) * 128],
                                src[k][:, off:off + n],
                                start=(k == 0),
                                stop=(k == KCH - 1),
                            )
                    for off, n in ntl:
                        out_cb(m, off, n, pss[off])

            def to_h(hs, b1t):
                def cb(m, off, n, ps):
                    nc.scalar.activation(hs[m][:, off:off + n], ps, Relu,
                                         bias=b1t[:, m:m + 1])
                return cb

            def to_y(ydram, b2t, ytag):
                def cb(m, off, n, ps):
                    yt = yp.tile([128, n], F32, tag=ytag, name=ytag)
                    nc.scalar.activation(yt, ps, Ident, bias=b2t[:, m:m + 1])
                    nc.sync.dma_start(
                        out=ydram[m * 128:(m + 1) * 128, off:off + n], in_=yt)
                return cb

            hs = alloc_kchunks(T_pad, "h", hp)
            hfs = alloc_kchunks(F_pad, "hf", hp)

            layer_blocked(xs, ws1, nt, to_h(hs, biases["b1"]))
            layer_wreuse(hs, ws2, nt, to_y(yT, biases["b2"], "y"))
            layer_blocked(xfs, wsf1, ntf, to_h(hfs, biases["bf1"]))
            layer_wreuse(hfs, wsf2, ntf, to_y(yfT, biases["bf2"], "yf"))

    _split_multi_waits(nc)
    return nc


_NC_CACHE = {}


def _get_nc(T_pad, F_pad):
    key = (T_pad, F_pad)
    if key not in _NC_CACHE:
        _NC_CACHE[key] = _build(T_pad, F_pad)
    return _NC_CACHE[key]


def _round_up(v, m):
    return ((v + m - 1) // m) * m


def kernel(x, W1, b1, W2, b2, Wf1, bf1, Wf2, bf2, routes, capacity,
           _trace=False):
    x = np.ascontiguousarray(np.asarray(x, dtype=np.float32))
    W1 = np.asarray(W1, dtype=np.float32)
    b1 = np.asarray(b1, dtype=np.float32)
    W2 = np.asarray(W2, dtype=np.float32)
    b2 = np.asarray(b2, dtype=np.float32)
    Wf1 = np.asarray(Wf1, dtype=np.float32)
    bf1 = np.asarray(bf1, dtype=np.float32)
    Wf2 = np.asarray(Wf2, dtype=np.float32)
    bf2 = np.asarray(bf2, dtype=np.float32)
    routes = np.asarray(routes)
    capacity = int(np.asarray(capacity))

    B, Dm = x.shape
    E = W1.shape[0]
    Kk = routes.shape[1]
    assert Dm == D and E == NCORES

    # --- routing: exact reference semantics (vectorized cumsum) ---
    e = routes.reshape(-1).astype(np.int64)
    valid = (e >= 0) & (e < E)
    e_safe = np.where(valid, e, 0)
    idx = np.arange(B * Kk)
    oh = np.zeros((B * Kk, E), dtype=np.int32)
    oh[idx[valid], e[valid]] = 1
    rank = np.cumsum(oh, axis=0) - oh
    rank_at = rank[idx, e_safe]
    accept_flat = valid & (rank_at < capacity)
    used = accept_flat.reshape(B, Kk).sum(1)

    # per-expert accepted assignment lists (flat order == reference order)
    tok_lists, fidx_lists, counts = [], [], []
    for el in range(E):
        fidx = np.nonzero(accept_flat & (e_safe == el))[0]
        fidx_lists.append(fidx)
        tok_lists.append(fidx // Kk)
        counts.append(len(fidx))
    T_pad = max(256, _round_up(max(counts), 256))
    src_flat = np.full(B * Kk, -1, dtype=np.int64)
    for el in range(E):
        src_flat[fidx_lists[el]] = el * T_pad + np.arange(counts[el])

    dropped = np.nonzero(used == 0)[0]
    F = len(dropped)
    Fc = max(1, -(-F // NCORES))
    F_pad = max(128, _round_up(Fc, 128))

    nc = _get_nc(T_pad, F_pad)

    # --- shard inputs ---
    def btile(v):
        return np.ascontiguousarray(v.reshape(KCH, 128).T)

    in_maps = []
    shared = {}
    for el in range(E):
        shared_el = {
            "w1T": np.ascontiguousarray(W1[el].T),
            "b1": btile(b1[el]),
            "w2T": np.ascontiguousarray(W2[el].T),
            "b2": btile(b2[el]),
        }
        if "wf1T" not in shared:
            shared["wf1T"] = np.ascontiguousarray(Wf1.T)
            shared["bf1"] = btile(bf1)
            shared["wf2T"] = np.ascontiguousarray(Wf2.T)
            shared["bf2"] = btile(bf2)

        toks = tok_lists[el]
        tpad = np.zeros(T_pad, dtype=np.int64)
        tpad[:len(toks)] = toks
        xTe = np.ascontiguousarray(x[tpad].T)

        lo, hi = el * Fc, min((el + 1) * Fc, F)
        fpad = np.zeros(F_pad, dtype=np.int64)
        if hi > lo:
            fpad[:hi - lo] = dropped[lo:hi]
        xfTe = np.ascontiguousarray(x[fpad].T)

        in_maps.append({"xT": xTe, "xfT": xfTe, **shared_el, **shared})

    res = run_bass_kernel_spmd(nc, in_maps, core_ids=list(range(NCORES)),
                               trace=_trace)

    # --- combine ---
    G = np.zeros((E * T_pad + 1, D), dtype=np.float32)
    for el in range(E):
        G[el * T_pad:(el + 1) * T_pad] = res.results[el]["yT"].T
    src = np.where(src_flat >= 0, src_flat, E * T_pad).reshape(B, Kk)
    summed = G[src].sum(axis=1)
    out = summed / np.maximum(used, 1.0).astype(np.float32)[:, None]

    if F > 0:
        fb_rows = np.empty((F, D), dtype=np.float32)
        for el in range(E):
            lo, hi = el * Fc, min((el + 1) * Fc, F)
            if hi > lo:
                fb_rows[lo:hi] = res.results[el]["yfT"].T[:hi - lo]
        out[dropped] = fb_rows

    if _trace:
        return out, res
    return out
